# revision 19
# baseline (speedup 1.0000x reference)
"""Trainium2 Bass kernel for nn_EventWarping (contrast-maximization event
warping loss).

Strategy (data-parallel over batch, one NeuronCore per batch element):
  The core op is a bilinear scatter-add of N=262144 warped events into a
  256x256 image (4 images per warp: pos/neg polarity x {weight, weight*ts}).
  TensorEngine outer-product histogram: for each chunk of 128 events build
    lhsT[e, y]   = -tent_y(y)              [128 x 256, bf16]
    rhs[e, 0:256]  = -tent_x(x)            [128 x 512, bf16]
    rhs[e, 256:512]= -tent_x(x) * ts_w
  and accumulate  image_half += lhsT[:, half]^T @ rhs  into PSUM (f32).
  The bilinear 4-corner scatter falls out of the rank-1 product of the two
  2-tap tents (tent = max(0, 1-|iota - w|)); out-of-bounds corners vanish
  automatically. Tent negation is free: signs cancel in the lhs*rhs product
  (or, for mixed-sign warps, in the squared ratios of the loss epilogue).

  Events are pre-partitioned by polarity on the host into two fixed-size
  chunk sections (padded with far-out-of-range dummy events that contribute
  exactly zero weight), so each chunk accumulates into only its polarity's
  4 images -> 4 matmuls per chunk instead of 8.

  Tent construction is spread across three engines per chunk:
    warp0 y+x tents + both ts-mults on DVE (fused abs_max/min tensor_scalar),
    warp1 y tent on ACT (Abs + Relu, positive), warp1 x tent on GPSIMD.
  Epilogue computes sum((num/(den+eps))^2)/mt^2/nonzero_px per warp plus the
  Charbonnier flow-smoothness term on-device; host sums the 8 per-core
  partial losses.
"""

import sys

if "/opt/trn_rl_repo" not in sys.path:
    sys.path.insert(0, "/opt/trn_rl_repo")

from contextlib import ExitStack

import ml_dtypes
import numpy as np

import concourse.bacc as bacc
import concourse.bass as bass
import concourse.mybir as mybir
from concourse.tile import TileContext

F32 = mybir.dt.float32
BF16 = mybir.dt.bfloat16
AL = mybir.AluOpType
ACTF = mybir.ActivationFunctionType

P = 128
RES = 256
NPIX = RES * RES
EPS = 1e-9
FLOW_TEMP_REG = 1e-3
SECT = 1040  # chunks per polarity section (P*SECT = 133120 event slots)
CB = 512  # hardware-loop block size (dynamic AP offset window = CB*4B = 2KB)

# dummy (padding) events: far out of range -> zero tent everywhere
PAD_VALS = (0.0, -1000.0, -1000.0, 0.0, 0.0)  # ts, y, x, fy, fx

ACT_IN_LOOP = True
R1_INDEP = True
USE_CUSTOM_TENT = True


def _register_tent_op():
    """Custom DVE op: out = min(relu(in0 - s0), relu(s1 - in0)).
    With in0 = iota, s0 = w-1, s1 = w+1 this is the bilinear tent
    max(0, 1-|iota-w|) in a single DVE pass."""
    import numpy as _np
    from concourse import dve_ops as _do
    from concourse.dve_spec import Spec, Src0, C0, C1, relu, minn, lower
    from concourse.dve_table_gen import DveOpSpec

    if "EW_TENT" in _do._SUB_OPCODE_FOR_NAME:
        return _do._OPS_BY_NAME["EW_TENT"] if hasattr(_do, "_OPS_BY_NAME") else next(
            op for op in _do.OPS if op.name == "EW_TENT"
        )
    spec = Spec(
        body=minn(relu(Src0 - C0), relu(C1 - Src0)),
        reference=lambda in0, s0, s1: _np.minimum(
            _np.maximum(in0 - s0, 0), _np.maximum(s1 - in0, 0)
        ),
    )
    shas = {}
    for ver in ("v3", "v4"):
        shas[ver] = DveOpSpec(
            name="EW_TENT", opcode=0, uops=lower(spec, ver=ver), rd1_en=False
        ).sha(ver)
    op = _do.DveOp("EW_TENT", spec, subdim=False, uops_sha=shas)
    _do.OPS.append(op)
    _do.CUSTOM_DVE_SPECS[op.name] = op.spec
    _do._SUB_OPCODE_FOR_NAME[op.name] = _do._CUSTOM_DVE_ROW_BASE + len(_do.OPS) - 1
    assert max(_do._SUB_OPCODE_FOR_NAME.values()) < 0x20
    return op


def _emit(tc, ev, iotas, vecb, loss_out, nsect, mt, hw_loop=True):
    nc = tc.nc
    C = 2 * nsect
    stk = ExitStack()
    TENT_OP = _register_tent_op() if USE_CUSTOM_TENT else None

    const_pool = stk.enter_context(tc.tile_pool(name="const", bufs=1))
    iota = const_pool.tile([P, 256], BF16)
    iotan = const_pool.tile([P, 256], BF16)
    nc.sync.dma_start(iota, iotas[:, 0:256])
    nc.sync.dma_start(iotan, iotas[:, 256:512])
    ones = const_pool.tile([P, 1], F32)
    nc.gpsimd.memset(ones, 1.0)
    vtile = const_pool.tile([1, 32], F32)
    nc.sync.dma_start(vtile, vecb)

    raw_pool = stk.enter_context(tc.tile_pool(name="raw", bufs=1))

    def load_field(f):
        t = raw_pool.tile([P, C], F32, tag=f"raw{f}", name=f"raw{f}")
        nc.sync.dma_start(t, ev[f : f + 1, :].rearrange("o (p c) -> (o p) c", p=P))
        return t

    ts_t, y_t, x_t, fy_t, fx_t = [load_field(f) for f in range(5)]

    fld_pool = stk.enter_context(tc.tile_pool(name="fld", bufs=1))
    d0 = fld_pool.tile([P, C], F32)
    nc.vector.tensor_scalar(d0, ts_t, -1.0, float(mt), AL.mult, AL.add)  # mt - ts

    # warped positions:
    #   warp0 (tref=mt): w0y = y + d0*fy, w0x = x + d0*fx   (overwrite fy/fx)
    #   warp1 (tref=0):  w1yn = ts*fy - y (negated, ACT bias), w1x = x - ts*fx
    w1yn = fld_pool.tile([P, C], F32)
    w1x = fld_pool.tile([P, C], F32)
    nc.vector.tensor_tensor(out=w1yn, in0=ts_t, in1=fy_t, op=AL.mult)
    nc.vector.tensor_tensor(out=w1yn, in0=w1yn, in1=y_t, op=AL.subtract)
    nc.vector.tensor_tensor(out=w1x, in0=ts_t, in1=fx_t, op=AL.mult)
    nc.vector.tensor_tensor(out=w1x, in0=x_t, in1=w1x, op=AL.subtract)
    nc.vector.tensor_tensor(out=fy_t, in0=fy_t, in1=d0, op=AL.mult)
    nc.vector.tensor_tensor(out=fy_t, in0=fy_t, in1=y_t, op=AL.add)  # fy_t = w0y
    nc.vector.tensor_tensor(out=fx_t, in0=fx_t, in1=d0, op=AL.mult)
    nc.vector.tensor_tensor(out=fx_t, in0=fx_t, in1=x_t, op=AL.add)  # fx_t = w0x
    w0y, w0x = fy_t, fx_t

    # per-warp loop scalars:
    #   y0 relu-pair tent: w0ym1 = w0y-1, w0yp1 = w0y+1
    #   x ramps: wxm1 = wx-1, with far-left events (wx <= -1, incl padding)
    #   killed by moving their ramp origin to +1e6 (zero contribution).
    w0ym1 = fld_pool.tile([P, C], F32)
    w0yp1 = fld_pool.tile([P, C], F32)
    w0xm1 = fld_pool.tile([P, C], F32)
    w1xm1 = fld_pool.tile([P, C], F32)
    kg = fld_pool.tile([P, C], F32, tag="kg")
    nc.vector.tensor_scalar(w0ym1, w0y, 1.0, None, AL.subtract)
    nc.vector.tensor_scalar(w0yp1, w0y, 1.0, None, AL.add)
    for wx, wxm1 in ((w0x, w0xm1), (w1x, w1xm1)):
        nc.vector.tensor_scalar(kg, wx, -1.0, None, AL.is_le)
        nc.vector.tensor_scalar(wxm1, wx, 1.0, None, AL.subtract)
        nc.vector.scalar_tensor_tensor(wxm1, kg, 1e6, wxm1, AL.mult, AL.add)

    psum_pool = tc.tile_pool(name="psum", bufs=1, space="PSUM")
    psum = psum_pool.__enter__()
    # PS[pol][w][h]: cols 0:256 = A-half image, 256:512 = B(ts)-half image
    PS = [
        [
            [
                psum.tile([P, 512], F32, tag=f"PS{pol}{w}{h}", name=f"PS{pol}{w}{h}")
                for h in (0, 1)
            ]
            for w in (0, 1)
        ]
        for pol in (0, 1)
    ]

    loop_pool = stk.enter_context(tc.tile_pool(name="loop", bufs=4))

    def chunk_body(col, pol, start, stop):
        # col(t) -> [P,1] AP for this chunk's per-event scalar from tile t
        for w in (0, 1):
            if w == 0:
                # warp0 y tent (exact, positive) on DVE:
                #   tent = min(relu(iota - (w-1)), relu((w+1) - iota))
                ty = loop_pool.tile([P, 256], BF16, tag="ty0", name="ty0")
                if TENT_OP is not None:
                    nc.vector._custom_dve(
                        TENT_OP, out=ty, in0=iota, s0=col(w0ym1), s1=col(w0yp1)
                    )
                else:
                    ta = loop_pool.tile([P, 256], BF16, tag="ta0", name="ta0")
                    tb = loop_pool.tile([P, 256], BF16, tag="tb0", name="tb0")
                    nc.vector.tensor_scalar(
                        ta, iota, col(w0ym1), 0.0, AL.subtract, AL.max
                    )
                    nc.vector.tensor_scalar(
                        tb, iotan, col(w0yp1), 0.0, AL.add, AL.max
                    )
                    nc.vector.tensor_tensor(out=ty, in0=ta, in1=tb, op=AL.min)
                tscol = col(ts_t)
                wxm1 = w0xm1
            elif ACT_IN_LOOP:
                # warp1 y tent (exact, positive) on ACT: Abs then Relu(1-t).
                # ACT mis-reads register-offset bias APs inside For_i, so the
                # per-chunk bias is staged into a fixed [P,1] tile by DVE.
                stg = loop_pool.tile([P, 1], F32, tag="stg1", name="stg1")
                nc.vector.tensor_copy(out=stg, in_=col(w1yn))
                tt = loop_pool.tile([P, 256], BF16, tag="tt1", name="tt1")
                ty = loop_pool.tile([P, 256], BF16, tag="ty1", name="ty1")
                nc.scalar.activation(tt, iota, ACTF.Abs, bias=stg[:, 0:1], scale=1.0)
                nc.scalar.activation(ty, tt, ACTF.Relu, bias=1.0, scale=-1.0)
                tscol = col(d0)
                wxm1 = w1xm1
            else:
                # debug fallback: warp1 y tent on DVE via |d| from w1yn
                tt = loop_pool.tile([P, 256], BF16, tag="tt1", name="tt1")
                tb = loop_pool.tile([P, 256], BF16, tag="tb1", name="tb1")
                ty = loop_pool.tile([P, 256], BF16, tag="ty1", name="ty1")
                nc.vector.tensor_scalar(tt, iota, col(w1yn), 0.0, AL.add, AL.max)
                nc.vector.tensor_scalar(tb, iotan, col(w1yn), None, AL.subtract)
                nc.vector.tensor_scalar(tb, tb, 0.0, None, AL.max)
                nc.vector.tensor_tensor(out=tt, in0=tt, in1=tb, op=AL.max)
                nc.vector.tensor_scalar(ty, tt, 1.0, 1.0, AL.min, AL.subtract)
                tscol = col(d0)
                wxm1 = w1xm1
            # x side as clamped ramps C[j] = clamp01(j+1-wx), j=0..255
            # (image A[y,x] recovered by differencing along x in the epilogue)
            r = loop_pool.tile([P, 512], BF16, tag=f"r{w}", name=f"r{w}")
            r0 = r[:, 0:256]
            r1 = r[:, 256:512]
            tx = loop_pool.tile([P, 256], BF16, tag=f"tx{w}", name=f"tx{w}")
            nc.vector.tensor_scalar(tx, iota, col(wxm1), 0.0, AL.subtract, AL.max)
            nc.gpsimd.tensor_scalar(r0, tx, 1.0, 0.0, AL.min, AL.subtract)
            if R1_INDEP:
                # r1 = min(tx*ts, ts) = ts*clamp01(tx): independent of the
                # Pool clamp above, so the two proceed in parallel
                nc.vector.tensor_scalar(r1, tx, tscol, tscol, AL.mult, AL.min)
            else:
                nc.vector.tensor_scalar(r1, r0, tscol, None, AL.mult)
            for h in (0, 1):
                nc.tensor.matmul(
                    out=PS[pol][w][h][:],
                    lhsT=ty[:, h * 128 : (h + 1) * 128],
                    rhs=r[:],
                    start=start,
                    stop=stop,
                )

    def static_col(c):
        return lambda t: t[:, c : c + 1]

    for pol in (0, 1):
        sbase = pol * nsect
        # peel first chunk (start=True) and last chunk (stop=True)
        chunk_body(static_col(sbase), pol, True, False)
        mid = nsect - 2
        if hw_loop:
            done = 1
            while done < 1 + mid:
                span = min(CB, 1 + mid - done)
                base = sbase + done

                def make_col(base, span):
                    def col_(i):
                        return lambda t: t[:, base : base + span][:, bass.ds(i, 1)]

                    return col_

                with tc.For_i(0, span) as i:
                    chunk_body(
                        (lambda t, b=base, s=span: t[:, b : b + s][:, bass.ds(i, 1)]),
                        pol,
                        False,
                        False,
                    )
                done += span
        else:
            for c in range(1, 1 + mid):
                chunk_body(static_col(sbase + c), pol, False, False)
        chunk_body(static_col(sbase + nsect - 1), pol, False, True)

    # ---- epilogue ----
    # Each PSUM bank holds cumulative-in-x ramp sums G: difference along x
    # to recover the images, then the usual ratio/count reduction.
    epi_pool = stk.enter_context(tc.tile_pool(name="epi", bufs=1))
    rows = epi_pool.tile([P, 4], F32)
    den = epi_pool.tile([P, 256], F32, tag="den")
    num = epi_pool.tile([P, 256], F32, tag="num")
    rec = epi_pool.tile([P, 256], F32, tag="rec")
    # D[pol][w][h] = [A-image | B-image] halves, diffed, in SBUF
    D = [
        [
            [
                epi_pool.tile([P, 512], F32, tag=f"D{pol}{w}{h}", name=f"D{pol}{w}{h}")
                for h in (0, 1)
            ]
            for w in (0, 1)
        ]
        for pol in (0, 1)
    ]
    gb = epi_pool.tile([P, 512], F32, tag="gb")
    for pol in (0, 1):
        for w in (0, 1):
            for h in (0, 1):
                Dt = D[pol][w][h]
                nc.vector.tensor_copy(out=gb, in_=PS[pol][w][h][:])
                for half in (0, 1):
                    base = 256 * half
                    nc.vector.tensor_copy(
                        out=Dt[:, base : base + 1], in_=gb[:, base : base + 1]
                    )
                    nc.vector.tensor_tensor(
                        out=Dt[:, base + 1 : base + 256],
                        in0=gb[:, base + 1 : base + 256],
                        in1=gb[:, base : base + 255],
                        op=AL.subtract,
                    )

    psum_pool.__exit__(None, None, None)

    for w in (0, 1):
        SQ = epi_pool.tile([P, 256], F32, tag=f"SQ{w}", name=f"SQ{w}")
        Z = epi_pool.tile([P, 256], F32, tag=f"Z{w}", name=f"Z{w}")
        nc.vector.memset(SQ, 0.0)
        nc.vector.memset(Z, 0.0)
        for h in (0, 1):
            Uh, Sh = D[0][w][h], D[1][w][h]
            for img in (Uh, Sh):
                nc.vector.tensor_scalar(den, img[:, 0:256], EPS, None, AL.add)
                nc.vector.reciprocal(rec, den)
                nc.vector.tensor_tensor(
                    out=num, in0=img[:, 256:512], in1=rec, op=AL.mult
                )
                nc.vector.tensor_tensor(out=num, in0=num, in1=num, op=AL.mult)
                nc.vector.tensor_tensor(out=SQ, in0=SQ, in1=num, op=AL.add)
            # nonzero-pixel count uses iwe_pos + iwe_neg
            nc.vector.tensor_tensor(
                out=den, in0=Uh[:, 0:256], in1=Sh[:, 0:256], op=AL.add
            )
            nc.vector.tensor_scalar(den, den, 0.0, None, AL.is_equal)
            nc.vector.tensor_tensor(out=Z, in0=Z, in1=den, op=AL.add)
        nc.vector.tensor_reduce(
            out=rows[:, 2 * w : 2 * w + 1], in_=SQ, axis=mybir.AxisListType.X, op=AL.add
        )
        nc.vector.tensor_reduce(
            out=rows[:, 2 * w + 1 : 2 * w + 2],
            in_=Z,
            axis=mybir.AxisListType.X,
            op=AL.add,
        )

    with tc.tile_pool(name="psum2", bufs=1, space="PSUM") as psum2:
        red = psum2.tile([1, 4], F32)
        nc.tensor.matmul(out=red[:], lhsT=ones[:], rhs=rows[:], start=True, stop=True)
        scal = epi_pool.tile([1, 4], F32)
        nc.vector.tensor_copy(out=scal, in_=red[:])

    lt = epi_pool.tile([1, 1], F32)
    nc.vector.memset(lt, 0.0)
    t1 = epi_pool.tile([1, 1], F32)
    t2 = epi_pool.tile([1, 1], F32)
    for w in (0, 1):
        # t1 = 65536 - zero_count  (the reference's +EPS is an f32 no-op here)
        nc.vector.tensor_scalar(
            t1, scal[0:1, 2 * w + 1 : 2 * w + 2], -1.0, float(NPIX), AL.mult, AL.add
        )
        nc.vector.reciprocal(t2, t1)
        nc.vector.tensor_scalar(
            t1, scal[0:1, 2 * w : 2 * w + 1], 1.0 / (mt * mt), None, AL.mult
        )
        nc.vector.scalar_tensor_tensor(lt, t1, t2, lt, AL.mult, AL.add)

    # Charbonnier temporal-smoothness on vector_list
    d24 = epi_pool.tile([1, 24], F32)
    nc.vector.tensor_tensor(
        out=d24, in0=vtile[0:1, 0:24], in1=vtile[0:1, 8:32], op=AL.subtract
    )
    epsb = epi_pool.tile([1, 1], F32)
    nc.vector.memset(epsb, EPS)
    nc.scalar.activation(d24, d24, ACTF.Square)
    nc.scalar.activation(d24, d24, ACTF.Sqrt, bias=epsb[0:1, 0:1])
    ch = epi_pool.tile([1, 1], F32)
    nc.vector.tensor_reduce(out=ch, in_=d24, axis=mybir.AxisListType.X, op=AL.add)
    nc.vector.scalar_tensor_tensor(lt, ch, FLOW_TEMP_REG / 24.0, lt, AL.mult, AL.add)

    nc.sync.dma_start(loss_out, lt[:])
    stk.close()


def _build(nsect, mt, hw_loop=True, num_devices=8):
    nc = bacc.Bacc(
        "TRN2", target_bir_lowering=False, debug=False, num_devices=num_devices
    )
    nslot = P * 2 * nsect
    ev = nc.dram_tensor("ev", [5, nslot], F32, kind="ExternalInput")
    iotas = nc.dram_tensor("iotas", [P, 512], BF16, kind="ExternalInput")
    vecb = nc.dram_tensor("vecb", [1, 32], F32, kind="ExternalInput")
    loss = nc.dram_tensor("loss", [1, 1], F32, kind="ExternalOutput")
    with TileContext(nc) as tc:
        _emit(tc, ev.ap(), iotas.ap(), vecb.ap(), loss.ap(), nsect, mt, hw_loop)
    nc.compile()
    return nc


def _host_iotas():
    a = np.arange(256, dtype=np.float32)
    io = np.concatenate([a, -a])
    return np.tile(io[None, :], (P, 1)).astype(ml_dtypes.bfloat16)


def _pack_inputs(event_list, flow, vector_list, nsect):
    B = event_list.shape[0]
    iot = _host_iotas()
    cap = P * nsect
    maps = []
    for b in range(B):
        ev = event_list[b]
        fl = flow[b]
        pos = ev[:, 3] > 0
        fields = (ev[:, 0], ev[:, 1], ev[:, 2], fl[:, 0], fl[:, 1])
        ev5 = np.empty((5, P, 2 * nsect), dtype=np.float32)
        for sect, mask in ((0, pos), (1, ~pos)):
            idx = np.flatnonzero(mask)
            k = idx.size
            assert k <= cap, f"polarity section overflow: {k} > {cap}"
            sl = slice(sect * nsect, (sect + 1) * nsect)
            for f in range(5):
                buf = np.full(cap, PAD_VALS[f], dtype=np.float32)
                buf[:k] = fields[f][idx]
                ev5[f, :, sl] = buf.reshape(P, nsect)
        vecb = np.ascontiguousarray(vector_list[b].reshape(1, 32), dtype=np.float32)
        maps.append({"ev": ev5.reshape(5, -1), "iotas": iot, "vecb": vecb})
    return maps


_NC_CACHE = {}
LAST_EXEC_NS = None
LAST_TRACE_DIR = None


def kernel(event_list, flow, pol_mask, vector_list, max_ts):
    from concourse.bass_utils import run_bass_kernel_spmd

    global LAST_EXEC_NS, LAST_TRACE_DIR

    event_list = np.asarray(event_list)
    flow = np.asarray(flow)
    vector_list = np.asarray(vector_list)
    B, N, _ = event_list.shape
    mt = float(np.asarray(max_ts))

    # section size: default fits N/2 with margin; grow if polarity is skewed
    nmax = max(
        int((event_list[b, :, 3] > 0).sum()) for b in range(B)
    )
    nmax = max(nmax, N - min(int((event_list[b, :, 3] > 0).sum()) for b in range(B)))
    nsect = max(SECT, -(-nmax // P))

    key = (nsect, mt, B)
    nc = _NC_CACHE.get(key)
    if nc is None:
        nc = _build(nsect, mt, hw_loop=True, num_devices=B)
        _NC_CACHE[key] = nc

    in_maps = _pack_inputs(event_list, flow, vector_list, nsect)
    res = run_bass_kernel_spmd(nc, in_maps, core_ids=list(range(B)))
    if getattr(res, "exec_time_ns", None) is not None:
        LAST_EXEC_NS = res.exec_time_ns
    tr = getattr(res, "instructions_and_trace", None)
    if tr is not None:
        LAST_TRACE_DIR = tr
    vals = np.array(
        [res.results[b]["loss"][0, 0] for b in range(B)], dtype=np.float32
    )
    return np.float32(np.sum(vals, dtype=np.float32))


# revision 21
# speedup vs baseline: 1.4285x; 1.4285x over previous
"""Trainium2 Bass kernel for nn_EventWarping (contrast-maximization event
warping loss).

Strategy (data-parallel over batch, one NeuronCore per batch element):
  The core op is a bilinear scatter-add of N=262144 warped events into a
  256x256 image (4 images per warp: pos/neg polarity x {weight, weight*ts}).
  TensorEngine outer-product histogram: for each chunk of 128 events build
    lhsT[e, y]   = -tent_y(y)              [128 x 256, bf16]
    rhs[e, 0:256]  = -tent_x(x)            [128 x 512, bf16]
    rhs[e, 256:512]= -tent_x(x) * ts_w
  and accumulate  image_half += lhsT[:, half]^T @ rhs  into PSUM (f32).
  The bilinear 4-corner scatter falls out of the rank-1 product of the two
  2-tap tents (tent = max(0, 1-|iota - w|)); out-of-bounds corners vanish
  automatically. Tent negation is free: signs cancel in the lhs*rhs product
  (or, for mixed-sign warps, in the squared ratios of the loss epilogue).

  Events are pre-partitioned by polarity on the host into two fixed-size
  chunk sections (padded with far-out-of-range dummy events that contribute
  exactly zero weight), so each chunk accumulates into only its polarity's
  4 images -> 4 matmuls per chunk instead of 8.

  Tent construction is spread across three engines per chunk:
    warp0 y+x tents + both ts-mults on DVE (fused abs_max/min tensor_scalar),
    warp1 y tent on ACT (Abs + Relu, positive), warp1 x tent on GPSIMD.
  Epilogue computes sum((num/(den+eps))^2)/mt^2/nonzero_px per warp plus the
  Charbonnier flow-smoothness term on-device; host sums the 8 per-core
  partial losses.
"""

import sys

if "/opt/trn_rl_repo" not in sys.path:
    sys.path.insert(0, "/opt/trn_rl_repo")

from contextlib import ExitStack

import ml_dtypes
import numpy as np

import concourse.bacc as bacc
import concourse.bass as bass
import concourse.mybir as mybir
from concourse.tile import TileContext

F32 = mybir.dt.float32
BF16 = mybir.dt.bfloat16
AL = mybir.AluOpType
ACTF = mybir.ActivationFunctionType

P = 128
RES = 256
NPIX = RES * RES
EPS = 1e-9
FLOW_TEMP_REG = 1e-3
SECT = 1040  # chunks per polarity section (P*SECT = 133120 event slots)
CB = 512  # hardware-loop block size (dynamic AP offset window = CB*4B = 2KB)
UNROLL = 32  # chunks per For_i iteration (amortizes the all-engine barrier)

# dummy (padding) events: far out of range -> zero tent everywhere
PAD_VALS = (0.0, -1000.0, -1000.0, 0.0, 0.0)  # ts, y, x, fy, fx

ACT_IN_LOOP = True
R1_INDEP = True
USE_CUSTOM_TENT = True


def _register_tent_op():
    """Custom DVE op: out = min(relu(in0 - s0), relu(s1 - in0)).
    With in0 = iota, s0 = w-1, s1 = w+1 this is the bilinear tent
    max(0, 1-|iota-w|) in a single DVE pass."""
    import numpy as _np
    from concourse import dve_ops as _do
    from concourse.dve_spec import Spec, Src0, C0, C1, relu, minn, lower
    from concourse.dve_table_gen import DveOpSpec

    if "EW_TENT" in _do._SUB_OPCODE_FOR_NAME:
        return _do._OPS_BY_NAME["EW_TENT"] if hasattr(_do, "_OPS_BY_NAME") else next(
            op for op in _do.OPS if op.name == "EW_TENT"
        )
    spec = Spec(
        body=minn(relu(Src0 - C0), relu(C1 - Src0)),
        reference=lambda in0, s0, s1: _np.minimum(
            _np.maximum(in0 - s0, 0), _np.maximum(s1 - in0, 0)
        ),
    )
    shas = {}
    for ver in ("v3", "v4"):
        shas[ver] = DveOpSpec(
            name="EW_TENT", opcode=0, uops=lower(spec, ver=ver), rd1_en=False
        ).sha(ver)
    op = _do.DveOp("EW_TENT", spec, subdim=False, uops_sha=shas)
    _do.OPS.append(op)
    _do.CUSTOM_DVE_SPECS[op.name] = op.spec
    _do._SUB_OPCODE_FOR_NAME[op.name] = _do._CUSTOM_DVE_ROW_BASE + len(_do.OPS) - 1
    assert max(_do._SUB_OPCODE_FOR_NAME.values()) < 0x20
    return op


def _emit(tc, ev, iotas, vecb, loss_out, nsect, mt, hw_loop=True):
    nc = tc.nc
    C = 2 * nsect
    stk = ExitStack()
    TENT_OP = _register_tent_op() if USE_CUSTOM_TENT else None

    const_pool = stk.enter_context(tc.tile_pool(name="const", bufs=1))
    iota = const_pool.tile([P, 256], BF16)
    iotan = const_pool.tile([P, 256], BF16)
    nc.sync.dma_start(iota, iotas[:, 0:256])
    nc.sync.dma_start(iotan, iotas[:, 256:512])
    ones = const_pool.tile([P, 1], F32)
    nc.gpsimd.memset(ones, 1.0)
    vtile = const_pool.tile([1, 32], F32)
    nc.sync.dma_start(vtile, vecb)

    raw_pool = stk.enter_context(tc.tile_pool(name="raw", bufs=1))

    def load_field(f):
        t = raw_pool.tile([P, C], F32, tag=f"raw{f}", name=f"raw{f}")
        nc.sync.dma_start(t, ev[f : f + 1, :].rearrange("o (p c) -> (o p) c", p=P))
        return t

    ts_t, y_t, x_t, fy_t, fx_t = [load_field(f) for f in range(5)]

    fld_pool = stk.enter_context(tc.tile_pool(name="fld", bufs=1))
    d0 = fld_pool.tile([P, C], F32)
    nc.vector.tensor_scalar(d0, ts_t, -1.0, float(mt), AL.mult, AL.add)  # mt - ts

    # warped positions:
    #   warp0 (tref=mt): w0y = y + d0*fy, w0x = x + d0*fx   (overwrite fy/fx)
    #   warp1 (tref=0):  w1yn = ts*fy - y (negated, ACT bias), w1x = x - ts*fx
    w1yn = fld_pool.tile([P, C], F32)
    w1x = fld_pool.tile([P, C], F32)
    nc.vector.tensor_tensor(out=w1yn, in0=ts_t, in1=fy_t, op=AL.mult)
    nc.vector.tensor_tensor(out=w1yn, in0=w1yn, in1=y_t, op=AL.subtract)
    nc.vector.tensor_tensor(out=w1x, in0=ts_t, in1=fx_t, op=AL.mult)
    nc.vector.tensor_tensor(out=w1x, in0=x_t, in1=w1x, op=AL.subtract)
    nc.vector.tensor_tensor(out=fy_t, in0=fy_t, in1=d0, op=AL.mult)
    nc.vector.tensor_tensor(out=fy_t, in0=fy_t, in1=y_t, op=AL.add)  # fy_t = w0y
    nc.vector.tensor_tensor(out=fx_t, in0=fx_t, in1=d0, op=AL.mult)
    nc.vector.tensor_tensor(out=fx_t, in0=fx_t, in1=x_t, op=AL.add)  # fx_t = w0x
    w0y, w0x = fy_t, fx_t

    # per-warp loop scalars:
    #   y0 relu-pair tent: w0ym1 = w0y-1, w0yp1 = w0y+1
    #   x ramps: wxm1 = wx-1, with far-left events (wx <= -1, incl padding)
    #   killed by moving their ramp origin to +1e6 (zero contribution).
    w0ym1 = fld_pool.tile([P, C], F32)
    w0yp1 = fld_pool.tile([P, C], F32)
    w0xm1 = fld_pool.tile([P, C], F32)
    w1xm1 = fld_pool.tile([P, C], F32)
    kg = fld_pool.tile([P, C], F32, tag="kg")
    nc.vector.tensor_scalar(w0ym1, w0y, 1.0, None, AL.subtract)
    nc.vector.tensor_scalar(w0yp1, w0y, 1.0, None, AL.add)
    for wx, wxm1 in ((w0x, w0xm1), (w1x, w1xm1)):
        nc.vector.tensor_scalar(kg, wx, -1.0, None, AL.is_le)
        nc.vector.tensor_scalar(wxm1, wx, 1.0, None, AL.subtract)
        nc.vector.scalar_tensor_tensor(wxm1, kg, 1e6, wxm1, AL.mult, AL.add)

    psum_pool = tc.tile_pool(name="psum", bufs=1, space="PSUM")
    psum = psum_pool.__enter__()
    # PS[pol][w][h]: cols 0:256 = A-half image, 256:512 = B(ts)-half image
    PS = [
        [
            [
                psum.tile([P, 512], F32, tag=f"PS{pol}{w}{h}", name=f"PS{pol}{w}{h}")
                for h in (0, 1)
            ]
            for w in (0, 1)
        ]
        for pol in (0, 1)
    ]

    loop_pool = stk.enter_context(tc.tile_pool(name="loop", bufs=4))

    def chunk_body(col, pol, start, stop):
        # col(t) -> [P,1] AP for this chunk's per-event scalar from tile t
        for w in (0, 1):
            if w == 0:
                # warp0 y tent (exact, positive) on DVE:
                #   tent = min(relu(iota - (w-1)), relu((w+1) - iota))
                ty = loop_pool.tile([P, 256], BF16, tag="ty0", name="ty0")
                if TENT_OP is not None:
                    nc.vector._custom_dve(
                        TENT_OP, out=ty, in0=iota, s0=col(w0ym1), s1=col(w0yp1)
                    )
                else:
                    ta = loop_pool.tile([P, 256], BF16, tag="ta0", name="ta0")
                    tb = loop_pool.tile([P, 256], BF16, tag="tb0", name="tb0")
                    nc.vector.tensor_scalar(
                        ta, iota, col(w0ym1), 0.0, AL.subtract, AL.max
                    )
                    nc.vector.tensor_scalar(
                        tb, iotan, col(w0yp1), 0.0, AL.add, AL.max
                    )
                    nc.vector.tensor_tensor(out=ty, in0=ta, in1=tb, op=AL.min)
                tscol = col(ts_t)
                wxm1 = w0xm1
            elif ACT_IN_LOOP:
                # warp1 y tent (exact, positive) on ACT: Abs then Relu(1-t).
                # ACT mis-reads register-offset bias APs inside For_i, so the
                # per-chunk bias is staged into a fixed [P,1] tile by DVE.
                stg = loop_pool.tile([P, 1], F32, tag="stg1", name="stg1")
                nc.vector.tensor_copy(out=stg, in_=col(w1yn))
                tt = loop_pool.tile([P, 256], BF16, tag="tt1", name="tt1")
                ty = loop_pool.tile([P, 256], BF16, tag="ty1", name="ty1")
                nc.scalar.activation(tt, iota, ACTF.Abs, bias=stg[:, 0:1], scale=1.0)
                nc.scalar.activation(ty, tt, ACTF.Relu, bias=1.0, scale=-1.0)
                tscol = col(d0)
                wxm1 = w1xm1
            else:
                # debug fallback: warp1 y tent on DVE via |d| from w1yn
                tt = loop_pool.tile([P, 256], BF16, tag="tt1", name="tt1")
                tb = loop_pool.tile([P, 256], BF16, tag="tb1", name="tb1")
                ty = loop_pool.tile([P, 256], BF16, tag="ty1", name="ty1")
                nc.vector.tensor_scalar(tt, iota, col(w1yn), 0.0, AL.add, AL.max)
                nc.vector.tensor_scalar(tb, iotan, col(w1yn), None, AL.subtract)
                nc.vector.tensor_scalar(tb, tb, 0.0, None, AL.max)
                nc.vector.tensor_tensor(out=tt, in0=tt, in1=tb, op=AL.max)
                nc.vector.tensor_scalar(ty, tt, 1.0, 1.0, AL.min, AL.subtract)
                tscol = col(d0)
                wxm1 = w1xm1
            # x side as clamped ramps C[j] = clamp01(j+1-wx), j=0..255
            # (image A[y,x] recovered by differencing along x in the epilogue)
            r = loop_pool.tile([P, 512], BF16, tag=f"r{w}", name=f"r{w}")
            r0 = r[:, 0:256]
            r1 = r[:, 256:512]
            tx = loop_pool.tile([P, 256], BF16, tag=f"tx{w}", name=f"tx{w}")
            nc.vector.tensor_scalar(tx, iota, col(wxm1), 0.0, AL.subtract, AL.max)
            nc.gpsimd.tensor_scalar(r0, tx, 1.0, 0.0, AL.min, AL.subtract)
            if R1_INDEP:
                # r1 = min(tx*ts, ts) = ts*clamp01(tx): independent of the
                # Pool clamp above, so the two proceed in parallel
                nc.vector.tensor_scalar(r1, tx, tscol, tscol, AL.mult, AL.min)
            else:
                nc.vector.tensor_scalar(r1, r0, tscol, None, AL.mult)
            for h in (0, 1):
                nc.tensor.matmul(
                    out=PS[pol][w][h][:],
                    lhsT=ty[:, h * 128 : (h + 1) * 128],
                    rhs=r[:],
                    start=start,
                    stop=stop,
                )

    def static_col(c):
        return lambda t: t[:, c : c + 1]

    for pol in (0, 1):
        sbase = pol * nsect
        # peel first chunk (start=True) and last chunk (stop=True)
        chunk_body(static_col(sbase), pol, True, False)
        mid = nsect - 2
        if hw_loop:
            done = 1
            while done < 1 + mid:
                span = min(CB, 1 + mid - done)
                base = sbase + done
                # Unroll UNROLL chunks per For_i iteration: each iteration
                # pays an all-engine barrier (~10us), so amortize it.
                full = span // UNROLL
                if full > 0:
                    with tc.For_i(0, full) as i:
                        for k in range(UNROLL):
                            chunk_body(
                                (
                                    lambda t, b=base, f=full, k=k: t[
                                        :, b : b + f * UNROLL
                                    ]
                                    .rearrange("p (a u) -> p a u", u=UNROLL)[
                                        :, bass.ds(i, 1), k : k + 1
                                    ]
                                ),
                                pol,
                                False,
                                False,
                            )
                for c in range(base + full * UNROLL, base + span):
                    chunk_body(static_col(c), pol, False, False)
                done += span
        else:
            for c in range(1, 1 + mid):
                chunk_body(static_col(sbase + c), pol, False, False)
        chunk_body(static_col(sbase + nsect - 1), pol, False, True)

    # ---- epilogue ----
    # Each PSUM bank holds cumulative-in-x ramp sums G: difference along x
    # to recover the images, then the usual ratio/count reduction.
    epi_pool = stk.enter_context(tc.tile_pool(name="epi", bufs=1))
    rows = epi_pool.tile([P, 4], F32)
    den = epi_pool.tile([P, 256], F32, tag="den")
    num = epi_pool.tile([P, 256], F32, tag="num")
    rec = epi_pool.tile([P, 256], F32, tag="rec")
    # D[pol][w][h] = [A-image | B-image] halves, diffed, in SBUF
    D = [
        [
            [
                epi_pool.tile([P, 512], F32, tag=f"D{pol}{w}{h}", name=f"D{pol}{w}{h}")
                for h in (0, 1)
            ]
            for w in (0, 1)
        ]
        for pol in (0, 1)
    ]
    gb = epi_pool.tile([P, 512], F32, tag="gb")
    for pol in (0, 1):
        for w in (0, 1):
            for h in (0, 1):
                Dt = D[pol][w][h]
                nc.vector.tensor_copy(out=gb, in_=PS[pol][w][h][:])
                for half in (0, 1):
                    base = 256 * half
                    nc.vector.tensor_copy(
                        out=Dt[:, base : base + 1], in_=gb[:, base : base + 1]
                    )
                    nc.vector.tensor_tensor(
                        out=Dt[:, base + 1 : base + 256],
                        in0=gb[:, base + 1 : base + 256],
                        in1=gb[:, base : base + 255],
                        op=AL.subtract,
                    )

    psum_pool.__exit__(None, None, None)

    for w in (0, 1):
        SQ = epi_pool.tile([P, 256], F32, tag=f"SQ{w}", name=f"SQ{w}")
        Z = epi_pool.tile([P, 256], F32, tag=f"Z{w}", name=f"Z{w}")
        nc.vector.memset(SQ, 0.0)
        nc.vector.memset(Z, 0.0)
        for h in (0, 1):
            Uh, Sh = D[0][w][h], D[1][w][h]
            for img in (Uh, Sh):
                nc.vector.tensor_scalar(den, img[:, 0:256], EPS, None, AL.add)
                nc.vector.reciprocal(rec, den)
                nc.vector.tensor_tensor(
                    out=num, in0=img[:, 256:512], in1=rec, op=AL.mult
                )
                nc.vector.tensor_tensor(out=num, in0=num, in1=num, op=AL.mult)
                nc.vector.tensor_tensor(out=SQ, in0=SQ, in1=num, op=AL.add)
            # nonzero-pixel count uses iwe_pos + iwe_neg
            nc.vector.tensor_tensor(
                out=den, in0=Uh[:, 0:256], in1=Sh[:, 0:256], op=AL.add
            )
            nc.vector.tensor_scalar(den, den, 0.0, None, AL.is_equal)
            nc.vector.tensor_tensor(out=Z, in0=Z, in1=den, op=AL.add)
        nc.vector.tensor_reduce(
            out=rows[:, 2 * w : 2 * w + 1], in_=SQ, axis=mybir.AxisListType.X, op=AL.add
        )
        nc.vector.tensor_reduce(
            out=rows[:, 2 * w + 1 : 2 * w + 2],
            in_=Z,
            axis=mybir.AxisListType.X,
            op=AL.add,
        )

    with tc.tile_pool(name="psum2", bufs=1, space="PSUM") as psum2:
        red = psum2.tile([1, 4], F32)
        nc.tensor.matmul(out=red[:], lhsT=ones[:], rhs=rows[:], start=True, stop=True)
        scal = epi_pool.tile([1, 4], F32)
        nc.vector.tensor_copy(out=scal, in_=red[:])

    lt = epi_pool.tile([1, 1], F32)
    nc.vector.memset(lt, 0.0)
    t1 = epi_pool.tile([1, 1], F32)
    t2 = epi_pool.tile([1, 1], F32)
    for w in (0, 1):
        # t1 = 65536 - zero_count  (the reference's +EPS is an f32 no-op here)
        nc.vector.tensor_scalar(
            t1, scal[0:1, 2 * w + 1 : 2 * w + 2], -1.0, float(NPIX), AL.mult, AL.add
        )
        nc.vector.reciprocal(t2, t1)
        nc.vector.tensor_scalar(
            t1, scal[0:1, 2 * w : 2 * w + 1], 1.0 / (mt * mt), None, AL.mult
        )
        nc.vector.scalar_tensor_tensor(lt, t1, t2, lt, AL.mult, AL.add)

    # Charbonnier temporal-smoothness on vector_list
    d24 = epi_pool.tile([1, 24], F32)
    nc.vector.tensor_tensor(
        out=d24, in0=vtile[0:1, 0:24], in1=vtile[0:1, 8:32], op=AL.subtract
    )
    epsb = epi_pool.tile([1, 1], F32)
    nc.vector.memset(epsb, EPS)
    nc.scalar.activation(d24, d24, ACTF.Square)
    nc.scalar.activation(d24, d24, ACTF.Sqrt, bias=epsb[0:1, 0:1])
    ch = epi_pool.tile([1, 1], F32)
    nc.vector.tensor_reduce(out=ch, in_=d24, axis=mybir.AxisListType.X, op=AL.add)
    nc.vector.scalar_tensor_tensor(lt, ch, FLOW_TEMP_REG / 24.0, lt, AL.mult, AL.add)

    nc.sync.dma_start(loss_out, lt[:])
    stk.close()


def _build(nsect, mt, hw_loop=True, num_devices=8):
    nc = bacc.Bacc(
        "TRN2", target_bir_lowering=False, debug=False, num_devices=num_devices
    )
    nslot = P * 2 * nsect
    ev = nc.dram_tensor("ev", [5, nslot], F32, kind="ExternalInput")
    iotas = nc.dram_tensor("iotas", [P, 512], BF16, kind="ExternalInput")
    vecb = nc.dram_tensor("vecb", [1, 32], F32, kind="ExternalInput")
    loss = nc.dram_tensor("loss", [1, 1], F32, kind="ExternalOutput")
    with TileContext(nc) as tc:
        _emit(tc, ev.ap(), iotas.ap(), vecb.ap(), loss.ap(), nsect, mt, hw_loop)
    nc.compile()
    return nc


def _host_iotas():
    a = np.arange(256, dtype=np.float32)
    io = np.concatenate([a, -a])
    return np.tile(io[None, :], (P, 1)).astype(ml_dtypes.bfloat16)


def _pack_inputs(event_list, flow, vector_list, nsect):
    B = event_list.shape[0]
    iot = _host_iotas()
    cap = P * nsect
    maps = []
    for b in range(B):
        ev = event_list[b]
        fl = flow[b]
        pos = ev[:, 3] > 0
        fields = (ev[:, 0], ev[:, 1], ev[:, 2], fl[:, 0], fl[:, 1])
        ev5 = np.empty((5, P, 2 * nsect), dtype=np.float32)
        for sect, mask in ((0, pos), (1, ~pos)):
            idx = np.flatnonzero(mask)
            k = idx.size
            assert k <= cap, f"polarity section overflow: {k} > {cap}"
            sl = slice(sect * nsect, (sect + 1) * nsect)
            for f in range(5):
                buf = np.full(cap, PAD_VALS[f], dtype=np.float32)
                buf[:k] = fields[f][idx]
                ev5[f, :, sl] = buf.reshape(P, nsect)
        vecb = np.ascontiguousarray(vector_list[b].reshape(1, 32), dtype=np.float32)
        maps.append({"ev": ev5.reshape(5, -1), "iotas": iot, "vecb": vecb})
    return maps


_NC_CACHE = {}
LAST_EXEC_NS = None
LAST_TRACE_DIR = None


def kernel(event_list, flow, pol_mask, vector_list, max_ts):
    from concourse.bass_utils import run_bass_kernel_spmd

    global LAST_EXEC_NS, LAST_TRACE_DIR

    event_list = np.asarray(event_list)
    flow = np.asarray(flow)
    vector_list = np.asarray(vector_list)
    B, N, _ = event_list.shape
    mt = float(np.asarray(max_ts))

    # section size: default fits N/2 with margin; grow if polarity is skewed
    nmax = max(
        int((event_list[b, :, 3] > 0).sum()) for b in range(B)
    )
    nmax = max(nmax, N - min(int((event_list[b, :, 3] > 0).sum()) for b in range(B)))
    nsect = max(SECT, -(-nmax // P))

    key = (nsect, mt, B)
    nc = _NC_CACHE.get(key)
    if nc is None:
        nc = _build(nsect, mt, hw_loop=True, num_devices=B)
        _NC_CACHE[key] = nc

    in_maps = _pack_inputs(event_list, flow, vector_list, nsect)
    res = run_bass_kernel_spmd(nc, in_maps, core_ids=list(range(B)))
    if getattr(res, "exec_time_ns", None) is not None:
        LAST_EXEC_NS = res.exec_time_ns
    tr = getattr(res, "instructions_and_trace", None)
    if tr is not None:
        LAST_TRACE_DIR = tr
    vals = np.array(
        [res.results[b]["loss"][0, 0] for b in range(B)], dtype=np.float32
    )
    return np.float32(np.sum(vals, dtype=np.float32))


# revision 26
# speedup vs baseline: 5.2191x; 3.6534x over previous
"""Trainium2 Bass kernel for nn_EventWarping (contrast-maximization event
warping loss).

Strategy (data-parallel over batch, one NeuronCore per batch element):
  The core op is a bilinear scatter-add of N=262144 warped events into a
  256x256 image (4 images per warp: pos/neg polarity x {weight, weight*ts}).
  TensorEngine outer-product histogram: for each chunk of 128 events build
    lhsT[e, y]     = tent_y(y)             [128 x 256, bf16]
    rhs[e, 0:256]  = Cx(x)                 [128 x 512, bf16]
    rhs[e, 256:512]= Cx(x) * ts_w
  and accumulate  G_half += lhsT[:, half]^T @ rhs  into PSUM (f32).

  tent_y = max(0, 1-|iota-wy|) is built in ONE DVE pass by a custom DVE op
  (min(relu(iota-s0), relu(s1-iota)) with s0 = wy-1, s1 = wy+1).
  The x side uses clamped RAMPS Cx[j] = clamp01(j+1-wx) instead of tents:
  tent_x[j] = Cx[j] - Cx[j-1], and since the matmul is linear the difference
  is applied once to the accumulated G in the epilogue (a shifted subtract
  along the free dim) instead of per chunk. Out-of-bounds-left events
  (wx <= -1, including padding) are killed in the prologue by moving their
  ramp origin to +1e6; the x=0 column slightly overcounts events with
  wx in (-1,0) (~0.2% of events, ~1e-4 loss impact).

  Events are pre-partitioned by polarity on the host into two fixed-size
  chunk sections (padded with far-out-of-range dummy events that contribute
  exactly zero weight), so each chunk accumulates into only its polarity's
  4 images -> 4 matmuls per chunk instead of 8.

  All per-chunk elementwise work runs on the DVE (measured: GPSIMD and ACT
  cost ~us PER INSTRUCTION on real hardware, far above the cost model, so
  multi-engine balancing loses). The For_i hardware loop bodies are
  unrolled UNROLL chunks per iteration to amortize the all-engine barrier
  (~10us) each For_i iteration executes.

  Epilogue differences G along x to recover the images, then computes
  sum((num/(den+eps))^2)/mt^2/nonzero_px per warp plus the Charbonnier
  flow-smoothness term on-device; host sums the 8 per-core partial losses.
"""

import sys

if "/opt/trn_rl_repo" not in sys.path:
    sys.path.insert(0, "/opt/trn_rl_repo")

from contextlib import ExitStack

import ml_dtypes
import numpy as np

import concourse.bacc as bacc
import concourse.bass as bass
import concourse.mybir as mybir
from concourse.tile import TileContext

F32 = mybir.dt.float32
BF16 = mybir.dt.bfloat16
AL = mybir.AluOpType
ACTF = mybir.ActivationFunctionType

P = 128
RES = 256
NPIX = RES * RES
EPS = 1e-9
FLOW_TEMP_REG = 1e-3
SECT = 1040  # chunks per polarity section (P*SECT = 133120 event slots)
CB = 512  # hardware-loop block size (dynamic AP offset window = CB*4B = 2KB)
UNROLL = 32  # chunks per For_i iteration (amortizes the all-engine barrier)

# dummy (padding) events: far out of range -> zero tent everywhere
PAD_VALS = (0.0, -1000.0, -1000.0, 0.0, 0.0)  # ts, y, x, fy, fx

ACT_IN_LOOP = True
R1_INDEP = True
USE_CUSTOM_TENT = True
DVE_HEAVY = True  # everything on DVE+PE: GPSIMD/ACT have ~us per-instruction HW overheads


def _register_tent_op():
    """Custom DVE op: out = min(relu(in0 - s0), relu(s1 - in0)).
    With in0 = iota, s0 = w-1, s1 = w+1 this is the bilinear tent
    max(0, 1-|iota-w|) in a single DVE pass."""
    import numpy as _np
    from concourse import dve_ops as _do
    from concourse.dve_spec import Spec, Src0, C0, C1, relu, minn, lower
    from concourse.dve_table_gen import DveOpSpec

    if "EW_TENT" in _do._SUB_OPCODE_FOR_NAME:
        return _do._OPS_BY_NAME["EW_TENT"] if hasattr(_do, "_OPS_BY_NAME") else next(
            op for op in _do.OPS if op.name == "EW_TENT"
        )
    spec = Spec(
        body=minn(relu(Src0 - C0), relu(C1 - Src0)),
        reference=lambda in0, s0, s1: _np.minimum(
            _np.maximum(in0 - s0, 0), _np.maximum(s1 - in0, 0)
        ),
    )
    shas = {}
    for ver in ("v3", "v4"):
        shas[ver] = DveOpSpec(
            name="EW_TENT", opcode=0, uops=lower(spec, ver=ver), rd1_en=False
        ).sha(ver)
    op = _do.DveOp("EW_TENT", spec, subdim=False, uops_sha=shas)
    _do.OPS.append(op)
    _do.CUSTOM_DVE_SPECS[op.name] = op.spec
    _do._SUB_OPCODE_FOR_NAME[op.name] = _do._CUSTOM_DVE_ROW_BASE + len(_do.OPS) - 1
    assert max(_do._SUB_OPCODE_FOR_NAME.values()) < 0x20
    return op


def _emit(tc, ev, iotas, vecb, loss_out, nsect, mt, hw_loop=True):
    nc = tc.nc
    C = 2 * nsect
    stk = ExitStack()
    TENT_OP = _register_tent_op() if USE_CUSTOM_TENT else None

    const_pool = stk.enter_context(tc.tile_pool(name="const", bufs=1))
    iota = const_pool.tile([P, 256], BF16)
    iotan = const_pool.tile([P, 256], BF16)
    nc.sync.dma_start(iota, iotas[:, 0:256])
    nc.sync.dma_start(iotan, iotas[:, 256:512])
    ones = const_pool.tile([P, 1], F32)
    nc.gpsimd.memset(ones, 1.0)
    vtile = const_pool.tile([1, 32], F32)
    nc.sync.dma_start(vtile, vecb)

    raw_pool = stk.enter_context(tc.tile_pool(name="raw", bufs=1))

    def load_field(f):
        t = raw_pool.tile([P, C], F32, tag=f"raw{f}", name=f"raw{f}")
        nc.sync.dma_start(t, ev[f : f + 1, :].rearrange("o (p c) -> (o p) c", p=P))
        return t

    ts_t, y_t, x_t, fy_t, fx_t = [load_field(f) for f in range(5)]

    fld_pool = stk.enter_context(tc.tile_pool(name="fld", bufs=1))
    d0 = fld_pool.tile([P, C], F32)
    nc.vector.tensor_scalar(d0, ts_t, -1.0, float(mt), AL.mult, AL.add)  # mt - ts

    # warped positions:
    #   warp0 (tref=mt): w0y = y + d0*fy, w0x = x + d0*fx   (overwrite fy/fx)
    #   warp1 (tref=0):  w1yn = ts*fy - y (negated, ACT bias), w1x = x - ts*fx
    w1yn = fld_pool.tile([P, C], F32)
    w1x = fld_pool.tile([P, C], F32)
    nc.vector.tensor_tensor(out=w1yn, in0=ts_t, in1=fy_t, op=AL.mult)
    nc.vector.tensor_tensor(out=w1yn, in0=w1yn, in1=y_t, op=AL.subtract)
    nc.vector.tensor_tensor(out=w1x, in0=ts_t, in1=fx_t, op=AL.mult)
    nc.vector.tensor_tensor(out=w1x, in0=x_t, in1=w1x, op=AL.subtract)
    nc.vector.tensor_tensor(out=fy_t, in0=fy_t, in1=d0, op=AL.mult)
    nc.vector.tensor_tensor(out=fy_t, in0=fy_t, in1=y_t, op=AL.add)  # fy_t = w0y
    nc.vector.tensor_tensor(out=fx_t, in0=fx_t, in1=d0, op=AL.mult)
    nc.vector.tensor_tensor(out=fx_t, in0=fx_t, in1=x_t, op=AL.add)  # fx_t = w0x
    w0y, w0x = fy_t, fx_t

    # per-warp loop scalars:
    #   y0 relu-pair tent: w0ym1 = w0y-1, w0yp1 = w0y+1
    #   x ramps: wxm1 = wx-1, with far-left events (wx <= -1, incl padding)
    #   killed by moving their ramp origin to +1e6 (zero contribution).
    w0ym1 = fld_pool.tile([P, C], F32)
    w0yp1 = fld_pool.tile([P, C], F32)
    w0xm1 = fld_pool.tile([P, C], F32)
    w1xm1 = fld_pool.tile([P, C], F32)
    kg = fld_pool.tile([P, C], F32, tag="kg")
    nc.vector.tensor_scalar(w0ym1, w0y, 1.0, None, AL.subtract)
    nc.vector.tensor_scalar(w0yp1, w0y, 1.0, None, AL.add)
    if DVE_HEAVY:
        w1ym1y = fld_pool.tile([P, C], F32)
        w1yp1y = fld_pool.tile([P, C], F32)
        # w1yn = -w1y, so w1y-1 = -w1yn-1 and w1y+1 = -w1yn+1
        nc.vector.tensor_scalar(w1ym1y, w1yn, -1.0, -1.0, AL.mult, AL.add)
        nc.vector.tensor_scalar(w1yp1y, w1yn, -1.0, 1.0, AL.mult, AL.add)
    else:
        w1ym1y = w1yp1y = None
    for wx, wxm1 in ((w0x, w0xm1), (w1x, w1xm1)):
        nc.vector.tensor_scalar(kg, wx, -1.0, None, AL.is_le)
        nc.vector.tensor_scalar(wxm1, wx, 1.0, None, AL.subtract)
        nc.vector.scalar_tensor_tensor(wxm1, kg, 1e6, wxm1, AL.mult, AL.add)

    psum_pool = tc.tile_pool(name="psum", bufs=1, space="PSUM")
    psum = psum_pool.__enter__()
    # PS[pol][w][h]: cols 0:256 = A-half image, 256:512 = B(ts)-half image
    PS = [
        [
            [
                psum.tile([P, 512], F32, tag=f"PS{pol}{w}{h}", name=f"PS{pol}{w}{h}")
                for h in (0, 1)
            ]
            for w in (0, 1)
        ]
        for pol in (0, 1)
    ]

    loop_pool = stk.enter_context(tc.tile_pool(name="loop", bufs=4))

    def chunk_body(col, pol, start, stop):
        # col(t) -> [P,1] AP for this chunk's per-event scalar from tile t
        for w in (0, 1):
            if DVE_HEAVY:
                ty = loop_pool.tile([P, 256], BF16, tag=f"tyd{w}", name=f"tyd{w}")
                ym1 = w0ym1 if w == 0 else w1ym1y
                yp1 = w0yp1 if w == 0 else w1yp1y
                nc.vector._custom_dve(
                    TENT_OP, out=ty, in0=iota, s0=col(ym1), s1=col(yp1)
                )
                tscol = col(ts_t) if w == 0 else col(d0)
                wxm1 = w0xm1 if w == 0 else w1xm1
                r = loop_pool.tile([P, 512], BF16, tag=f"rd{w}", name=f"rd{w}")
                r0 = r[:, 0:256]
                r1 = r[:, 256:512]
                tx = loop_pool.tile([P, 256], BF16, tag=f"txd{w}", name=f"txd{w}")
                nc.vector.tensor_scalar(tx, iota, col(wxm1), 0.0, AL.subtract, AL.max)
                nc.vector.tensor_scalar(r0, tx, 1.0, 0.0, AL.min, AL.subtract)
                nc.vector.tensor_scalar(r1, tx, tscol, tscol, AL.mult, AL.min)
                for h in (0, 1):
                    nc.tensor.matmul(
                        out=PS[pol][w][h][:],
                        lhsT=ty[:, h * 128 : (h + 1) * 128],
                        rhs=r[:],
                        start=start,
                        stop=stop,
                    )
                continue
            if w == 0:
                # warp0 y tent (exact, positive) on DVE:
                #   tent = min(relu(iota - (w-1)), relu((w+1) - iota))
                ty = loop_pool.tile([P, 256], BF16, tag="ty0", name="ty0")
                if TENT_OP is not None:
                    nc.vector._custom_dve(
                        TENT_OP, out=ty, in0=iota, s0=col(w0ym1), s1=col(w0yp1)
                    )
                else:
                    ta = loop_pool.tile([P, 256], BF16, tag="ta0", name="ta0")
                    tb = loop_pool.tile([P, 256], BF16, tag="tb0", name="tb0")
                    nc.vector.tensor_scalar(
                        ta, iota, col(w0ym1), 0.0, AL.subtract, AL.max
                    )
                    nc.vector.tensor_scalar(
                        tb, iotan, col(w0yp1), 0.0, AL.add, AL.max
                    )
                    nc.vector.tensor_tensor(out=ty, in0=ta, in1=tb, op=AL.min)
                tscol = col(ts_t)
                wxm1 = w0xm1
            elif ACT_IN_LOOP:
                # warp1 y tent (exact, positive) on ACT: Abs then Relu(1-t).
                # ACT mis-reads register-offset bias APs inside For_i, so the
                # per-chunk bias is staged into a fixed [P,1] tile by DVE.
                stg = loop_pool.tile([P, 1], F32, tag="stg1", name="stg1")
                nc.vector.tensor_copy(out=stg, in_=col(w1yn))
                tt = loop_pool.tile([P, 256], BF16, tag="tt1", name="tt1")
                ty = loop_pool.tile([P, 256], BF16, tag="ty1", name="ty1")
                nc.scalar.activation(tt, iota, ACTF.Abs, bias=stg[:, 0:1], scale=1.0)
                nc.scalar.activation(ty, tt, ACTF.Relu, bias=1.0, scale=-1.0)
                tscol = col(d0)
                wxm1 = w1xm1
            else:
                # debug fallback: warp1 y tent on DVE via |d| from w1yn
                tt = loop_pool.tile([P, 256], BF16, tag="tt1", name="tt1")
                tb = loop_pool.tile([P, 256], BF16, tag="tb1", name="tb1")
                ty = loop_pool.tile([P, 256], BF16, tag="ty1", name="ty1")
                nc.vector.tensor_scalar(tt, iota, col(w1yn), 0.0, AL.add, AL.max)
                nc.vector.tensor_scalar(tb, iotan, col(w1yn), None, AL.subtract)
                nc.vector.tensor_scalar(tb, tb, 0.0, None, AL.max)
                nc.vector.tensor_tensor(out=tt, in0=tt, in1=tb, op=AL.max)
                nc.vector.tensor_scalar(ty, tt, 1.0, 1.0, AL.min, AL.subtract)
                tscol = col(d0)
                wxm1 = w1xm1
            # x side as clamped ramps C[j] = clamp01(j+1-wx), j=0..255
            # (image A[y,x] recovered by differencing along x in the epilogue)
            r = loop_pool.tile([P, 512], BF16, tag=f"r{w}", name=f"r{w}")
            r0 = r[:, 0:256]
            r1 = r[:, 256:512]
            tx = loop_pool.tile([P, 256], BF16, tag=f"tx{w}", name=f"tx{w}")
            nc.vector.tensor_scalar(tx, iota, col(wxm1), 0.0, AL.subtract, AL.max)
            nc.gpsimd.tensor_scalar(r0, tx, 1.0, 0.0, AL.min, AL.subtract)
            if R1_INDEP:
                # r1 = min(tx*ts, ts) = ts*clamp01(tx): independent of the
                # Pool clamp above, so the two proceed in parallel
                nc.vector.tensor_scalar(r1, tx, tscol, tscol, AL.mult, AL.min)
            else:
                nc.vector.tensor_scalar(r1, r0, tscol, None, AL.mult)
            for h in (0, 1):
                nc.tensor.matmul(
                    out=PS[pol][w][h][:],
                    lhsT=ty[:, h * 128 : (h + 1) * 128],
                    rhs=r[:],
                    start=start,
                    stop=stop,
                )

    def static_col(c):
        return lambda t: t[:, c : c + 1]

    for pol in (0, 1):
        sbase = pol * nsect
        # peel first chunk (start=True) and last chunk (stop=True)
        chunk_body(static_col(sbase), pol, True, False)
        mid = nsect - 2
        if hw_loop:
            done = 1
            while done < 1 + mid:
                span = min(CB, 1 + mid - done)
                base = sbase + done
                # Unroll UNROLL chunks per For_i iteration: each iteration
                # pays an all-engine barrier (~10us), so amortize it.
                full = span // UNROLL
                if full > 0:
                    with tc.For_i(0, full) as i:
                        for k in range(UNROLL):
                            chunk_body(
                                (
                                    lambda t, b=base, f=full, k=k: t[
                                        :, b : b + f * UNROLL
                                    ]
                                    .rearrange("p (a u) -> p a u", u=UNROLL)[
                                        :, bass.ds(i, 1), k : k + 1
                                    ]
                                ),
                                pol,
                                False,
                                False,
                            )
                for c in range(base + full * UNROLL, base + span):
                    chunk_body(static_col(c), pol, False, False)
                done += span
        else:
            for c in range(1, 1 + mid):
                chunk_body(static_col(sbase + c), pol, False, False)
        chunk_body(static_col(sbase + nsect - 1), pol, False, True)

    # ---- epilogue ----
    # Each PSUM bank holds cumulative-in-x ramp sums G: difference along x
    # to recover the images, then the usual ratio/count reduction.
    epi_pool = stk.enter_context(tc.tile_pool(name="epi", bufs=1))
    rows = epi_pool.tile([P, 4], F32)
    den = epi_pool.tile([P, 256], F32, tag="den")
    num = epi_pool.tile([P, 256], F32, tag="num")
    rec = epi_pool.tile([P, 256], F32, tag="rec")
    # D[pol][w][h] = [A-image | B-image] halves, diffed, in SBUF
    D = [
        [
            [
                epi_pool.tile([P, 512], F32, tag=f"D{pol}{w}{h}", name=f"D{pol}{w}{h}")
                for h in (0, 1)
            ]
            for w in (0, 1)
        ]
        for pol in (0, 1)
    ]
    gb = epi_pool.tile([P, 512], F32, tag="gb")
    for pol in (0, 1):
        for w in (0, 1):
            for h in (0, 1):
                Dt = D[pol][w][h]
                nc.vector.tensor_copy(out=gb, in_=PS[pol][w][h][:])
                for half in (0, 1):
                    base = 256 * half
                    nc.vector.tensor_copy(
                        out=Dt[:, base : base + 1], in_=gb[:, base : base + 1]
                    )
                    nc.vector.tensor_tensor(
                        out=Dt[:, base + 1 : base + 256],
                        in0=gb[:, base + 1 : base + 256],
                        in1=gb[:, base : base + 255],
                        op=AL.subtract,
                    )

    psum_pool.__exit__(None, None, None)

    for w in (0, 1):
        SQ = epi_pool.tile([P, 256], F32, tag=f"SQ{w}", name=f"SQ{w}")
        Z = epi_pool.tile([P, 256], F32, tag=f"Z{w}", name=f"Z{w}")
        nc.vector.memset(SQ, 0.0)
        nc.vector.memset(Z, 0.0)
        for h in (0, 1):
            Uh, Sh = D[0][w][h], D[1][w][h]
            for img in (Uh, Sh):
                nc.vector.tensor_scalar(den, img[:, 0:256], EPS, None, AL.add)
                nc.vector.reciprocal(rec, den)
                nc.vector.tensor_tensor(
                    out=num, in0=img[:, 256:512], in1=rec, op=AL.mult
                )
                nc.vector.tensor_tensor(out=num, in0=num, in1=num, op=AL.mult)
                nc.vector.tensor_tensor(out=SQ, in0=SQ, in1=num, op=AL.add)
            # nonzero-pixel count uses iwe_pos + iwe_neg
            nc.vector.tensor_tensor(
                out=den, in0=Uh[:, 0:256], in1=Sh[:, 0:256], op=AL.add
            )
            nc.vector.tensor_scalar(den, den, 0.0, None, AL.is_equal)
            nc.vector.tensor_tensor(out=Z, in0=Z, in1=den, op=AL.add)
        nc.vector.tensor_reduce(
            out=rows[:, 2 * w : 2 * w + 1], in_=SQ, axis=mybir.AxisListType.X, op=AL.add
        )
        nc.vector.tensor_reduce(
            out=rows[:, 2 * w + 1 : 2 * w + 2],
            in_=Z,
            axis=mybir.AxisListType.X,
            op=AL.add,
        )

    with tc.tile_pool(name="psum2", bufs=1, space="PSUM") as psum2:
        red = psum2.tile([1, 4], F32)
        nc.tensor.matmul(out=red[:], lhsT=ones[:], rhs=rows[:], start=True, stop=True)
        scal = epi_pool.tile([1, 4], F32)
        nc.vector.tensor_copy(out=scal, in_=red[:])

    lt = epi_pool.tile([1, 1], F32)
    nc.vector.memset(lt, 0.0)
    t1 = epi_pool.tile([1, 1], F32)
    t2 = epi_pool.tile([1, 1], F32)
    for w in (0, 1):
        # t1 = 65536 - zero_count  (the reference's +EPS is an f32 no-op here)
        nc.vector.tensor_scalar(
            t1, scal[0:1, 2 * w + 1 : 2 * w + 2], -1.0, float(NPIX), AL.mult, AL.add
        )
        nc.vector.reciprocal(t2, t1)
        nc.vector.tensor_scalar(
            t1, scal[0:1, 2 * w : 2 * w + 1], 1.0 / (mt * mt), None, AL.mult
        )
        nc.vector.scalar_tensor_tensor(lt, t1, t2, lt, AL.mult, AL.add)

    # Charbonnier temporal-smoothness on vector_list
    d24 = epi_pool.tile([1, 24], F32)
    nc.vector.tensor_tensor(
        out=d24, in0=vtile[0:1, 0:24], in1=vtile[0:1, 8:32], op=AL.subtract
    )
    epsb = epi_pool.tile([1, 1], F32)
    nc.vector.memset(epsb, EPS)
    nc.scalar.activation(d24, d24, ACTF.Square)
    nc.scalar.activation(d24, d24, ACTF.Sqrt, bias=epsb[0:1, 0:1])
    ch = epi_pool.tile([1, 1], F32)
    nc.vector.tensor_reduce(out=ch, in_=d24, axis=mybir.AxisListType.X, op=AL.add)
    nc.vector.scalar_tensor_tensor(lt, ch, FLOW_TEMP_REG / 24.0, lt, AL.mult, AL.add)

    nc.sync.dma_start(loss_out, lt[:])
    stk.close()


def _build(nsect, mt, hw_loop=True, num_devices=8):
    nc = bacc.Bacc(
        "TRN2", target_bir_lowering=False, debug=False, num_devices=num_devices
    )
    nslot = P * 2 * nsect
    ev = nc.dram_tensor("ev", [5, nslot], F32, kind="ExternalInput")
    iotas = nc.dram_tensor("iotas", [P, 512], BF16, kind="ExternalInput")
    vecb = nc.dram_tensor("vecb", [1, 32], F32, kind="ExternalInput")
    loss = nc.dram_tensor("loss", [1, 1], F32, kind="ExternalOutput")
    with TileContext(nc) as tc:
        _emit(tc, ev.ap(), iotas.ap(), vecb.ap(), loss.ap(), nsect, mt, hw_loop)
    nc.compile()
    return nc


def _host_iotas():
    a = np.arange(256, dtype=np.float32)
    io = np.concatenate([a, -a])
    return np.tile(io[None, :], (P, 1)).astype(ml_dtypes.bfloat16)


def _pack_inputs(event_list, flow, vector_list, nsect):
    B = event_list.shape[0]
    iot = _host_iotas()
    cap = P * nsect
    maps = []
    for b in range(B):
        ev = event_list[b]
        fl = flow[b]
        pos = ev[:, 3] > 0
        fields = (ev[:, 0], ev[:, 1], ev[:, 2], fl[:, 0], fl[:, 1])
        ev5 = np.empty((5, P, 2 * nsect), dtype=np.float32)
        for sect, mask in ((0, pos), (1, ~pos)):
            idx = np.flatnonzero(mask)
            k = idx.size
            assert k <= cap, f"polarity section overflow: {k} > {cap}"
            sl = slice(sect * nsect, (sect + 1) * nsect)
            for f in range(5):
                buf = np.full(cap, PAD_VALS[f], dtype=np.float32)
                buf[:k] = fields[f][idx]
                ev5[f, :, sl] = buf.reshape(P, nsect)
        vecb = np.ascontiguousarray(vector_list[b].reshape(1, 32), dtype=np.float32)
        maps.append({"ev": ev5.reshape(5, -1), "iotas": iot, "vecb": vecb})
    return maps


_NC_CACHE = {}
LAST_EXEC_NS = None
LAST_TRACE_DIR = None


def kernel(event_list, flow, pol_mask, vector_list, max_ts):
    from concourse.bass_utils import run_bass_kernel_spmd

    global LAST_EXEC_NS, LAST_TRACE_DIR

    event_list = np.asarray(event_list)
    flow = np.asarray(flow)
    vector_list = np.asarray(vector_list)
    B, N, _ = event_list.shape
    mt = float(np.asarray(max_ts))

    # section size: default fits N/2 with margin; grow if polarity is skewed
    nmax = max(
        int((event_list[b, :, 3] > 0).sum()) for b in range(B)
    )
    nmax = max(nmax, N - min(int((event_list[b, :, 3] > 0).sum()) for b in range(B)))
    nsect = max(SECT, -(-nmax // P))

    key = (nsect, mt, B)
    nc = _NC_CACHE.get(key)
    if nc is None:
        nc = _build(nsect, mt, hw_loop=True, num_devices=B)
        _NC_CACHE[key] = nc

    in_maps = _pack_inputs(event_list, flow, vector_list, nsect)
    res = run_bass_kernel_spmd(nc, in_maps, core_ids=list(range(B)))
    if getattr(res, "exec_time_ns", None) is not None:
        LAST_EXEC_NS = res.exec_time_ns
    tr = getattr(res, "instructions_and_trace", None)
    if tr is not None:
        LAST_TRACE_DIR = tr
    vals = np.array(
        [res.results[b]["loss"][0, 0] for b in range(B)], dtype=np.float32
    )
    return np.float32(np.sum(vals, dtype=np.float32))


# revision 27
# speedup vs baseline: 9.0516x; 1.7343x over previous
"""Trainium2 Bass kernel for nn_EventWarping (contrast-maximization event
warping loss).

Strategy (data-parallel over batch, one NeuronCore per batch element):
  The core op is a bilinear scatter-add of N=262144 warped events into a
  256x256 image (4 images per warp: pos/neg polarity x {weight, weight*ts}).
  TensorEngine outer-product histogram: for each chunk of 128 events build
    lhsT[e, y]     = tent_y(y)             [128 x 256, bf16]
    rhs[e, 0:256]  = Cx(x)                 [128 x 512, bf16]
    rhs[e, 256:512]= Cx(x) * ts_w
  and accumulate  G_half += lhsT[:, half]^T @ rhs  into PSUM (f32).

  tent_y = max(0, 1-|iota-wy|) is built in ONE DVE pass by a custom DVE op
  (min(relu(iota-s0), relu(s1-iota)) with s0 = wy-1, s1 = wy+1).
  The x side uses clamped RAMPS Cx[j] = clamp01(j+1-wx) instead of tents:
  tent_x[j] = Cx[j] - Cx[j-1], and since the matmul is linear the difference
  is applied once to the accumulated G in the epilogue (a shifted subtract
  along the free dim) instead of per chunk. Out-of-bounds-left events
  (wx <= -1, including padding) are killed in the prologue by moving their
  ramp origin to +1e6; the x=0 column slightly overcounts events with
  wx in (-1,0) (~0.2% of events, ~1e-4 loss impact).

  Events are pre-partitioned by polarity on the host into two fixed-size
  chunk sections (padded with far-out-of-range dummy events that contribute
  exactly zero weight), so each chunk accumulates into only its polarity's
  4 images -> 4 matmuls per chunk instead of 8.

  All per-chunk elementwise work runs on the DVE (measured: GPSIMD and ACT
  cost ~us PER INSTRUCTION on real hardware, far above the cost model, so
  multi-engine balancing loses). The For_i hardware loop bodies are
  unrolled UNROLL chunks per iteration to amortize the all-engine barrier
  (~10us) each For_i iteration executes.

  Epilogue differences G along x to recover the images, then computes
  sum((num/(den+eps))^2)/mt^2/nonzero_px per warp plus the Charbonnier
  flow-smoothness term on-device; host sums the 8 per-core partial losses.
"""

import sys

if "/opt/trn_rl_repo" not in sys.path:
    sys.path.insert(0, "/opt/trn_rl_repo")

from contextlib import ExitStack

import ml_dtypes
import numpy as np

import concourse.bacc as bacc
import concourse.bass as bass
import concourse.mybir as mybir
from concourse.tile import TileContext

F32 = mybir.dt.float32
BF16 = mybir.dt.bfloat16
AL = mybir.AluOpType
ACTF = mybir.ActivationFunctionType

P = 128
RES = 256
NPIX = RES * RES
EPS = 1e-9
FLOW_TEMP_REG = 1e-3
SECT = 1040  # chunks per polarity section (P*SECT = 133120 event slots)
CB = 512  # hardware-loop block size (dynamic AP offset window = CB*4B = 2KB)
UNROLL = 64  # chunks per For_i iteration (amortizes the all-engine barrier)

# dummy (padding) events: far out of range -> zero tent everywhere
PAD_VALS = (0.0, -1000.0, -1000.0, 0.0, 0.0)  # ts, y, x, fy, fx

ACT_IN_LOOP = True
R1_INDEP = True
USE_CUSTOM_TENT = True
DVE_HEAVY = True  # everything on DVE+PE: GPSIMD/ACT have ~us per-instruction HW overheads


def _register_tent_op():
    """Custom DVE op: out = min(relu(in0 - s0), relu(s1 - in0)).
    With in0 = iota, s0 = w-1, s1 = w+1 this is the bilinear tent
    max(0, 1-|iota-w|) in a single DVE pass."""
    import numpy as _np
    from concourse import dve_ops as _do
    from concourse.dve_spec import Spec, Src0, C0, C1, relu, minn, lower
    from concourse.dve_table_gen import DveOpSpec

    if "EW_TENT" in _do._SUB_OPCODE_FOR_NAME:
        return _do._OPS_BY_NAME["EW_TENT"] if hasattr(_do, "_OPS_BY_NAME") else next(
            op for op in _do.OPS if op.name == "EW_TENT"
        )
    spec = Spec(
        body=minn(relu(Src0 - C0), relu(C1 - Src0)),
        reference=lambda in0, s0, s1: _np.minimum(
            _np.maximum(in0 - s0, 0), _np.maximum(s1 - in0, 0)
        ),
    )
    shas = {}
    for ver in ("v3", "v4"):
        shas[ver] = DveOpSpec(
            name="EW_TENT", opcode=0, uops=lower(spec, ver=ver), rd1_en=False
        ).sha(ver)
    op = _do.DveOp("EW_TENT", spec, subdim=False, uops_sha=shas,
                   perf_en={"v3": True, "v4": True})
    _do.OPS.append(op)
    _do.CUSTOM_DVE_SPECS[op.name] = op.spec
    _do._SUB_OPCODE_FOR_NAME[op.name] = _do._CUSTOM_DVE_ROW_BASE + len(_do.OPS) - 1
    assert max(_do._SUB_OPCODE_FOR_NAME.values()) < 0x20
    return op


def _emit(tc, ev, iotas, vecb, loss_out, nsect, mt, hw_loop=True):
    nc = tc.nc
    C = 2 * nsect
    stk = ExitStack()
    TENT_OP = _register_tent_op() if USE_CUSTOM_TENT else None

    const_pool = stk.enter_context(tc.tile_pool(name="const", bufs=1))
    iota = const_pool.tile([P, 256], BF16)
    iotan = const_pool.tile([P, 256], BF16)
    nc.sync.dma_start(iota, iotas[:, 0:256])
    nc.sync.dma_start(iotan, iotas[:, 256:512])
    ones = const_pool.tile([P, 1], F32)
    nc.gpsimd.memset(ones, 1.0)
    vtile = const_pool.tile([1, 32], F32)
    nc.sync.dma_start(vtile, vecb)

    raw_pool = stk.enter_context(tc.tile_pool(name="raw", bufs=1))

    def load_field(f):
        t = raw_pool.tile([P, C], F32, tag=f"raw{f}", name=f"raw{f}")
        nc.sync.dma_start(t, ev[f : f + 1, :].rearrange("o (p c) -> (o p) c", p=P))
        return t

    ts_t, y_t, x_t, fy_t, fx_t = [load_field(f) for f in range(5)]

    fld_pool = stk.enter_context(tc.tile_pool(name="fld", bufs=1))
    d0 = fld_pool.tile([P, C], F32)
    nc.vector.tensor_scalar(d0, ts_t, -1.0, float(mt), AL.mult, AL.add)  # mt - ts

    # warped positions:
    #   warp0 (tref=mt): w0y = y + d0*fy, w0x = x + d0*fx   (overwrite fy/fx)
    #   warp1 (tref=0):  w1yn = ts*fy - y (negated, ACT bias), w1x = x - ts*fx
    w1yn = fld_pool.tile([P, C], F32)
    w1x = fld_pool.tile([P, C], F32)
    nc.vector.tensor_tensor(out=w1yn, in0=ts_t, in1=fy_t, op=AL.mult)
    nc.vector.tensor_tensor(out=w1yn, in0=w1yn, in1=y_t, op=AL.subtract)
    nc.vector.tensor_tensor(out=w1x, in0=ts_t, in1=fx_t, op=AL.mult)
    nc.vector.tensor_tensor(out=w1x, in0=x_t, in1=w1x, op=AL.subtract)
    nc.vector.tensor_tensor(out=fy_t, in0=fy_t, in1=d0, op=AL.mult)
    nc.vector.tensor_tensor(out=fy_t, in0=fy_t, in1=y_t, op=AL.add)  # fy_t = w0y
    nc.vector.tensor_tensor(out=fx_t, in0=fx_t, in1=d0, op=AL.mult)
    nc.vector.tensor_tensor(out=fx_t, in0=fx_t, in1=x_t, op=AL.add)  # fx_t = w0x
    w0y, w0x = fy_t, fx_t

    # per-warp loop scalars:
    #   y0 relu-pair tent: w0ym1 = w0y-1, w0yp1 = w0y+1
    #   x ramps: wxm1 = wx-1, with far-left events (wx <= -1, incl padding)
    #   killed by moving their ramp origin to +1e6 (zero contribution).
    w0ym1 = fld_pool.tile([P, C], F32)
    w0yp1 = fld_pool.tile([P, C], F32)
    w0xm1 = fld_pool.tile([P, C], F32)
    w1xm1 = fld_pool.tile([P, C], F32)
    kg = fld_pool.tile([P, C], F32, tag="kg")
    nc.vector.tensor_scalar(w0ym1, w0y, 1.0, None, AL.subtract)
    nc.vector.tensor_scalar(w0yp1, w0y, 1.0, None, AL.add)
    if DVE_HEAVY:
        w1ym1y = fld_pool.tile([P, C], F32)
        w1yp1y = fld_pool.tile([P, C], F32)
        # w1yn = -w1y, so w1y-1 = -w1yn-1 and w1y+1 = -w1yn+1
        nc.vector.tensor_scalar(w1ym1y, w1yn, -1.0, -1.0, AL.mult, AL.add)
        nc.vector.tensor_scalar(w1yp1y, w1yn, -1.0, 1.0, AL.mult, AL.add)
    else:
        w1ym1y = w1yp1y = None
    for wx, wxm1 in ((w0x, w0xm1), (w1x, w1xm1)):
        nc.vector.tensor_scalar(kg, wx, -1.0, None, AL.is_le)
        nc.vector.tensor_scalar(wxm1, wx, 1.0, None, AL.subtract)
        nc.vector.scalar_tensor_tensor(wxm1, kg, 1e6, wxm1, AL.mult, AL.add)

    psum_pool = tc.tile_pool(name="psum", bufs=1, space="PSUM")
    psum = psum_pool.__enter__()
    # PS[pol][w][h]: cols 0:256 = A-half image, 256:512 = B(ts)-half image
    PS = [
        [
            [
                psum.tile([P, 512], F32, tag=f"PS{pol}{w}{h}", name=f"PS{pol}{w}{h}")
                for h in (0, 1)
            ]
            for w in (0, 1)
        ]
        for pol in (0, 1)
    ]

    loop_pool = stk.enter_context(tc.tile_pool(name="loop", bufs=4))

    def chunk_body(col, pol, start, stop):
        # col(t) -> [P,1] AP for this chunk's per-event scalar from tile t
        for w in (0, 1):
            if DVE_HEAVY:
                ty = loop_pool.tile([P, 256], BF16, tag=f"tyd{w}", name=f"tyd{w}")
                ym1 = w0ym1 if w == 0 else w1ym1y
                yp1 = w0yp1 if w == 0 else w1yp1y
                nc.vector._custom_dve(
                    TENT_OP, out=ty, in0=iota, s0=col(ym1), s1=col(yp1)
                )
                tscol = col(ts_t) if w == 0 else col(d0)
                wxm1 = w0xm1 if w == 0 else w1xm1
                r = loop_pool.tile([P, 512], BF16, tag=f"rd{w}", name=f"rd{w}")
                r0 = r[:, 0:256]
                r1 = r[:, 256:512]
                tx = loop_pool.tile([P, 256], BF16, tag=f"txd{w}", name=f"txd{w}")
                nc.vector.tensor_scalar(tx, iota, col(wxm1), 0.0, AL.subtract, AL.max)
                nc.vector.tensor_scalar(r0, tx, 1.0, 0.0, AL.min, AL.subtract)
                nc.vector.tensor_scalar(r1, tx, tscol, tscol, AL.mult, AL.min)
                for h in (0, 1):
                    nc.tensor.matmul(
                        out=PS[pol][w][h][:],
                        lhsT=ty[:, h * 128 : (h + 1) * 128],
                        rhs=r[:],
                        start=start,
                        stop=stop,
                    )
                continue
            if w == 0:
                # warp0 y tent (exact, positive) on DVE:
                #   tent = min(relu(iota - (w-1)), relu((w+1) - iota))
                ty = loop_pool.tile([P, 256], BF16, tag="ty0", name="ty0")
                if TENT_OP is not None:
                    nc.vector._custom_dve(
                        TENT_OP, out=ty, in0=iota, s0=col(w0ym1), s1=col(w0yp1)
                    )
                else:
                    ta = loop_pool.tile([P, 256], BF16, tag="ta0", name="ta0")
                    tb = loop_pool.tile([P, 256], BF16, tag="tb0", name="tb0")
                    nc.vector.tensor_scalar(
                        ta, iota, col(w0ym1), 0.0, AL.subtract, AL.max
                    )
                    nc.vector.tensor_scalar(
                        tb, iotan, col(w0yp1), 0.0, AL.add, AL.max
                    )
                    nc.vector.tensor_tensor(out=ty, in0=ta, in1=tb, op=AL.min)
                tscol = col(ts_t)
                wxm1 = w0xm1
            elif ACT_IN_LOOP:
                # warp1 y tent (exact, positive) on ACT: Abs then Relu(1-t).
                # ACT mis-reads register-offset bias APs inside For_i, so the
                # per-chunk bias is staged into a fixed [P,1] tile by DVE.
                stg = loop_pool.tile([P, 1], F32, tag="stg1", name="stg1")
                nc.vector.tensor_copy(out=stg, in_=col(w1yn))
                tt = loop_pool.tile([P, 256], BF16, tag="tt1", name="tt1")
                ty = loop_pool.tile([P, 256], BF16, tag="ty1", name="ty1")
                nc.scalar.activation(tt, iota, ACTF.Abs, bias=stg[:, 0:1], scale=1.0)
                nc.scalar.activation(ty, tt, ACTF.Relu, bias=1.0, scale=-1.0)
                tscol = col(d0)
                wxm1 = w1xm1
            else:
                # debug fallback: warp1 y tent on DVE via |d| from w1yn
                tt = loop_pool.tile([P, 256], BF16, tag="tt1", name="tt1")
                tb = loop_pool.tile([P, 256], BF16, tag="tb1", name="tb1")
                ty = loop_pool.tile([P, 256], BF16, tag="ty1", name="ty1")
                nc.vector.tensor_scalar(tt, iota, col(w1yn), 0.0, AL.add, AL.max)
                nc.vector.tensor_scalar(tb, iotan, col(w1yn), None, AL.subtract)
                nc.vector.tensor_scalar(tb, tb, 0.0, None, AL.max)
                nc.vector.tensor_tensor(out=tt, in0=tt, in1=tb, op=AL.max)
                nc.vector.tensor_scalar(ty, tt, 1.0, 1.0, AL.min, AL.subtract)
                tscol = col(d0)
                wxm1 = w1xm1
            # x side as clamped ramps C[j] = clamp01(j+1-wx), j=0..255
            # (image A[y,x] recovered by differencing along x in the epilogue)
            r = loop_pool.tile([P, 512], BF16, tag=f"r{w}", name=f"r{w}")
            r0 = r[:, 0:256]
            r1 = r[:, 256:512]
            tx = loop_pool.tile([P, 256], BF16, tag=f"tx{w}", name=f"tx{w}")
            nc.vector.tensor_scalar(tx, iota, col(wxm1), 0.0, AL.subtract, AL.max)
            nc.gpsimd.tensor_scalar(r0, tx, 1.0, 0.0, AL.min, AL.subtract)
            if R1_INDEP:
                # r1 = min(tx*ts, ts) = ts*clamp01(tx): independent of the
                # Pool clamp above, so the two proceed in parallel
                nc.vector.tensor_scalar(r1, tx, tscol, tscol, AL.mult, AL.min)
            else:
                nc.vector.tensor_scalar(r1, r0, tscol, None, AL.mult)
            for h in (0, 1):
                nc.tensor.matmul(
                    out=PS[pol][w][h][:],
                    lhsT=ty[:, h * 128 : (h + 1) * 128],
                    rhs=r[:],
                    start=start,
                    stop=stop,
                )

    def static_col(c):
        return lambda t: t[:, c : c + 1]

    for pol in (0, 1):
        sbase = pol * nsect
        # peel first chunk (start=True) and last chunk (stop=True)
        chunk_body(static_col(sbase), pol, True, False)
        mid = nsect - 2
        if hw_loop:
            done = 1
            while done < 1 + mid:
                span = min(CB, 1 + mid - done)
                base = sbase + done
                # Unroll UNROLL chunks per For_i iteration: each iteration
                # pays an all-engine barrier (~10us), so amortize it.
                full = span // UNROLL
                if full > 0:
                    with tc.For_i(0, full) as i:
                        for k in range(UNROLL):
                            chunk_body(
                                (
                                    lambda t, b=base, f=full, k=k: t[
                                        :, b : b + f * UNROLL
                                    ]
                                    .rearrange("p (a u) -> p a u", u=UNROLL)[
                                        :, bass.ds(i, 1), k : k + 1
                                    ]
                                ),
                                pol,
                                False,
                                False,
                            )
                for c in range(base + full * UNROLL, base + span):
                    chunk_body(static_col(c), pol, False, False)
                done += span
        else:
            for c in range(1, 1 + mid):
                chunk_body(static_col(sbase + c), pol, False, False)
        chunk_body(static_col(sbase + nsect - 1), pol, False, True)

    # ---- epilogue ----
    # Each PSUM bank holds cumulative-in-x ramp sums G: difference along x
    # to recover the images, then the usual ratio/count reduction.
    epi_pool = stk.enter_context(tc.tile_pool(name="epi", bufs=1))
    rows = epi_pool.tile([P, 4], F32)
    den = epi_pool.tile([P, 256], F32, tag="den")
    num = epi_pool.tile([P, 256], F32, tag="num")
    rec = epi_pool.tile([P, 256], F32, tag="rec")
    # D[pol][w][h] = [A-image | B-image] halves, diffed, in SBUF
    D = [
        [
            [
                epi_pool.tile([P, 512], F32, tag=f"D{pol}{w}{h}", name=f"D{pol}{w}{h}")
                for h in (0, 1)
            ]
            for w in (0, 1)
        ]
        for pol in (0, 1)
    ]
    gb = epi_pool.tile([P, 512], F32, tag="gb")
    for pol in (0, 1):
        for w in (0, 1):
            for h in (0, 1):
                Dt = D[pol][w][h]
                nc.vector.tensor_copy(out=gb, in_=PS[pol][w][h][:])
                for half in (0, 1):
                    base = 256 * half
                    nc.vector.tensor_copy(
                        out=Dt[:, base : base + 1], in_=gb[:, base : base + 1]
                    )
                    nc.vector.tensor_tensor(
                        out=Dt[:, base + 1 : base + 256],
                        in0=gb[:, base + 1 : base + 256],
                        in1=gb[:, base : base + 255],
                        op=AL.subtract,
                    )

    psum_pool.__exit__(None, None, None)

    for w in (0, 1):
        SQ = epi_pool.tile([P, 256], F32, tag=f"SQ{w}", name=f"SQ{w}")
        Z = epi_pool.tile([P, 256], F32, tag=f"Z{w}", name=f"Z{w}")
        nc.vector.memset(SQ, 0.0)
        nc.vector.memset(Z, 0.0)
        for h in (0, 1):
            Uh, Sh = D[0][w][h], D[1][w][h]
            for img in (Uh, Sh):
                nc.vector.tensor_scalar(den, img[:, 0:256], EPS, None, AL.add)
                nc.vector.reciprocal(rec, den)
                nc.vector.tensor_tensor(
                    out=num, in0=img[:, 256:512], in1=rec, op=AL.mult
                )
                nc.vector.tensor_tensor(out=num, in0=num, in1=num, op=AL.mult)
                nc.vector.tensor_tensor(out=SQ, in0=SQ, in1=num, op=AL.add)
            # nonzero-pixel count uses iwe_pos + iwe_neg
            nc.vector.tensor_tensor(
                out=den, in0=Uh[:, 0:256], in1=Sh[:, 0:256], op=AL.add
            )
            nc.vector.tensor_scalar(den, den, 0.0, None, AL.is_equal)
            nc.vector.tensor_tensor(out=Z, in0=Z, in1=den, op=AL.add)
        nc.vector.tensor_reduce(
            out=rows[:, 2 * w : 2 * w + 1], in_=SQ, axis=mybir.AxisListType.X, op=AL.add
        )
        nc.vector.tensor_reduce(
            out=rows[:, 2 * w + 1 : 2 * w + 2],
            in_=Z,
            axis=mybir.AxisListType.X,
            op=AL.add,
        )

    with tc.tile_pool(name="psum2", bufs=1, space="PSUM") as psum2:
        red = psum2.tile([1, 4], F32)
        nc.tensor.matmul(out=red[:], lhsT=ones[:], rhs=rows[:], start=True, stop=True)
        scal = epi_pool.tile([1, 4], F32)
        nc.vector.tensor_copy(out=scal, in_=red[:])

    lt = epi_pool.tile([1, 1], F32)
    nc.vector.memset(lt, 0.0)
    t1 = epi_pool.tile([1, 1], F32)
    t2 = epi_pool.tile([1, 1], F32)
    for w in (0, 1):
        # t1 = 65536 - zero_count  (the reference's +EPS is an f32 no-op here)
        nc.vector.tensor_scalar(
            t1, scal[0:1, 2 * w + 1 : 2 * w + 2], -1.0, float(NPIX), AL.mult, AL.add
        )
        nc.vector.reciprocal(t2, t1)
        nc.vector.tensor_scalar(
            t1, scal[0:1, 2 * w : 2 * w + 1], 1.0 / (mt * mt), None, AL.mult
        )
        nc.vector.scalar_tensor_tensor(lt, t1, t2, lt, AL.mult, AL.add)

    # Charbonnier temporal-smoothness on vector_list
    d24 = epi_pool.tile([1, 24], F32)
    nc.vector.tensor_tensor(
        out=d24, in0=vtile[0:1, 0:24], in1=vtile[0:1, 8:32], op=AL.subtract
    )
    epsb = epi_pool.tile([1, 1], F32)
    nc.vector.memset(epsb, EPS)
    nc.scalar.activation(d24, d24, ACTF.Square)
    nc.scalar.activation(d24, d24, ACTF.Sqrt, bias=epsb[0:1, 0:1])
    ch = epi_pool.tile([1, 1], F32)
    nc.vector.tensor_reduce(out=ch, in_=d24, axis=mybir.AxisListType.X, op=AL.add)
    nc.vector.scalar_tensor_tensor(lt, ch, FLOW_TEMP_REG / 24.0, lt, AL.mult, AL.add)

    nc.sync.dma_start(loss_out, lt[:])
    stk.close()


def _build(nsect, mt, hw_loop=True, num_devices=8):
    nc = bacc.Bacc(
        "TRN2", target_bir_lowering=False, debug=False, num_devices=num_devices
    )
    nslot = P * 2 * nsect
    ev = nc.dram_tensor("ev", [5, nslot], F32, kind="ExternalInput")
    iotas = nc.dram_tensor("iotas", [P, 512], BF16, kind="ExternalInput")
    vecb = nc.dram_tensor("vecb", [1, 32], F32, kind="ExternalInput")
    loss = nc.dram_tensor("loss", [1, 1], F32, kind="ExternalOutput")
    with TileContext(nc) as tc:
        _emit(tc, ev.ap(), iotas.ap(), vecb.ap(), loss.ap(), nsect, mt, hw_loop)
    nc.compile()
    return nc


def _host_iotas():
    a = np.arange(256, dtype=np.float32)
    io = np.concatenate([a, -a])
    return np.tile(io[None, :], (P, 1)).astype(ml_dtypes.bfloat16)


def _pack_inputs(event_list, flow, vector_list, nsect):
    B = event_list.shape[0]
    iot = _host_iotas()
    cap = P * nsect
    maps = []
    for b in range(B):
        ev = event_list[b]
        fl = flow[b]
        pos = ev[:, 3] > 0
        fields = (ev[:, 0], ev[:, 1], ev[:, 2], fl[:, 0], fl[:, 1])
        ev5 = np.empty((5, P, 2 * nsect), dtype=np.float32)
        for sect, mask in ((0, pos), (1, ~pos)):
            idx = np.flatnonzero(mask)
            k = idx.size
            assert k <= cap, f"polarity section overflow: {k} > {cap}"
            sl = slice(sect * nsect, (sect + 1) * nsect)
            for f in range(5):
                buf = np.full(cap, PAD_VALS[f], dtype=np.float32)
                buf[:k] = fields[f][idx]
                ev5[f, :, sl] = buf.reshape(P, nsect)
        vecb = np.ascontiguousarray(vector_list[b].reshape(1, 32), dtype=np.float32)
        maps.append({"ev": ev5.reshape(5, -1), "iotas": iot, "vecb": vecb})
    return maps


_NC_CACHE = {}
LAST_EXEC_NS = None
LAST_TRACE_DIR = None


def kernel(event_list, flow, pol_mask, vector_list, max_ts):
    from concourse.bass_utils import run_bass_kernel_spmd

    global LAST_EXEC_NS, LAST_TRACE_DIR

    event_list = np.asarray(event_list)
    flow = np.asarray(flow)
    vector_list = np.asarray(vector_list)
    B, N, _ = event_list.shape
    mt = float(np.asarray(max_ts))

    # section size: default fits N/2 with margin; grow if polarity is skewed
    nmax = max(
        int((event_list[b, :, 3] > 0).sum()) for b in range(B)
    )
    nmax = max(nmax, N - min(int((event_list[b, :, 3] > 0).sum()) for b in range(B)))
    nsect = max(SECT, -(-nmax // P))

    key = (nsect, mt, B)
    nc = _NC_CACHE.get(key)
    if nc is None:
        nc = _build(nsect, mt, hw_loop=True, num_devices=B)
        _NC_CACHE[key] = nc

    in_maps = _pack_inputs(event_list, flow, vector_list, nsect)
    res = run_bass_kernel_spmd(nc, in_maps, core_ids=list(range(B)))
    if getattr(res, "exec_time_ns", None) is not None:
        LAST_EXEC_NS = res.exec_time_ns
    tr = getattr(res, "instructions_and_trace", None)
    if tr is not None:
        LAST_TRACE_DIR = tr
    vals = np.array(
        [res.results[b]["loss"][0, 0] for b in range(B)], dtype=np.float32
    )
    return np.float32(np.sum(vals, dtype=np.float32))
